# Initial kernel scaffold
#
"""Trainium2 Bass kernel for nn_EstraNet_1443109012284.

Mathematical reduction: the reference's FAVOR+/trig branch (phi_q, aux_q/k,
fr_q/k, aux_A, A) does not feed the output.  The output is exactly

    out[b,n,d] = sum_{h,c} W_o[h,c,d] * norma[h] * sum_{d'} W_v[d',h,c] * x[b,n,d']
               = (x @ M)[b,n,d],   M[d',d] = sum_{h,c} W_v[d',h,c] norma[h] W_o[h,c,d]

with norma[h] = || sum_d s_p[h] W_p[d,h,:] beta_p[d] ||_2.

M is a tiny [512,512] matrix folded on the host; the device does the single
big GEMM  y[32768,512] = x[32768,512] @ M[512,512]  data-parallel over rows:
each of the 8 cores handles 4096 rows.

Device design (per core): compute yT[d, n] = sum_k M[k,d] xT[k,n]
- fp16; loop h(4 quarters) -> k(4 chunks) -> d(4) -> j(2); 8 PSUM banks
  (d,j) per quarter accumulated over k; M pre-scaled by a power of two.
- Measured constraints this schedule is built around:
  * per-HWDGE-ring input rate ~180-250 GB/s, ~360-400 aggregate;
  * DMA completion sem fires ~1.3-4us after last byte (HBM receipt,
    load-dependent) -> first data-gated MM lands ~4.6us;
  * fixed ~2.6us to first DMA packet and a fixed ~7us epilogue
    (255-semaphore clear sweep) on every kernel;
  * PE stream floor 128 x 216ns = 27.7us; any PE idle gap re-throttles
    the HAM clock to half rate.
- m[k0] (sync) and x[h0,k0] (scalar) race as the two leading DMAs, one
  receipt each in parallel; the other m slices and h0 chunks interleave
  across both rings in need order (m1+h0k1+h0k3 on sync, m23+h0k2 on
  scalar); later quarters alternate rings in consumption order.
- Warmup MMs (128-wide, memset-gated) hold the PE from ~1us to the first
  data sem, burning the HAM cold-clock ramp; overshoot costs 107ns each.
- PSUM->SBUF drains split by bank parity: ACT copies j0 banks, DVE j1
  (parallel PSUM readers on different banks are fine on TRN2).
- k-outer for h0-h2 (quarter starts after one chunk), d-outer for the
  last quarter (inputs all resident; spreads the final 1MB of output
  instead of bunching it after the last MM).
- Outputs blocked [h,d,128,1024] (each store DRAM-contiguous); early
  stores on gpsimd SWDGE, later ones on sync/scalar after their input
  queues drain; final tile's two [128,512] halves store on sync and
  scalar in parallel so the tail is one copy + one small store + receipt.
"""

import os as _os
import sys

sys.path.insert(0, "/opt/trn_rl_repo")

import numpy as np

import concourse.bass as bass
import concourse.tile as tile
from concourse import bacc, mybir
from concourse.bass_utils import run_bass_kernel_spmd

N_CORES = 8
ROWS = 32768           # B*N = 8*4096
RPC = ROWS // N_CORES  # rows per core = 4096
D = 512
KC = 4                 # contraction chunks of 128
DT = D // 128          # output row-blocks = 4
HB = 4                 # n-quarters per stripe
HW = RPC // HB         # 1024 columns per quarter
JH = HW // 512         # moving chunks of 512 per quarter = 2

COMPUTE_DTYPE = _os.environ.get("KERNEL_DTYPE", "fp16")
N_WARM = int(_os.environ.get("KERNEL_NWARM", "33"))
WARM_MEMSET = _os.environ.get("KERNEL_WARM_MEMSET", "1") == "1"

_DT = {
    "fp32": mybir.dt.float32,
    "f32r": mybir.dt.float32r,
    "bf16": mybir.dt.bfloat16,
    "fp16": mybir.dt.float16,
}


def _np_dtype(token):
    if token == "bf16":
        import ml_dtypes

        return ml_dtypes.bfloat16
    if token == "fp16":
        return np.float16
    return np.float32


def _build(token):
    dt_in = _DT[token]
    dt_out = mybir.dt.float16 if token == "fp16" else mybir.dt.float32
    nc = bacc.Bacc("TRN2", target_bir_lowering=False)
    xt0 = nc.dram_tensor("xt0", [KC, 128, HW], dt_in, kind="ExternalInput")
    xq = nc.dram_tensor("xq", [HB - 1, 128, KC * HW], dt_in, kind="ExternalInput")
    mm = nc.dram_tensor("mm", [128, KC, D], dt_in, kind="ExternalInput")
    # output blocked [h, d, 128, 1024] so each store is DRAM-contiguous
    yt = nc.dram_tensor("yt", [HB, DT, 128, HW], dt_out, kind="ExternalOutput")

    with tile.TileContext(nc) as tc:
        with (
            tc.tile_pool(name="xp", bufs=1) as xp,
            tc.tile_pool(name="mp", bufs=1) as mp,
            tc.tile_pool(name="op", bufs=4) as op,
            tc.tile_pool(name="pp", bufs=8, space="PSUM") as pp,
        ):
            # PE warmup: 128-wide matmuls (on uninitialized SBUF - contents
            # are irrelevant, the PSUM bank is cleared by the first real MM)
            # bridge the PE from ~0.4us to the first data sem and burn the
            # HAM cold-clock ramp; each overshoot MM costs only ~107ns.
            wz = mp.tile([128, 128], mybir.dt.float16, name="wz")
            if WARM_MEMSET:
                nc.gpsimd.memset(wz[:], 1.0)
            warm = pp.tile([128, 512], mybir.dt.float32, tag="ps", name="warm")
            for w in range(N_WARM):
                nc.tensor.matmul(
                    warm[:, 0:128], wz[:], wz[:], start=True, stop=True
                )

            # two parallel critical DMAs lead the rings: full m (sync) and
            # x[h0,k0] (scalar) — each sem pays one ~2-3us receipt, in
            # parallel.  Then chunks alternate rings in consumption order:
            # odd k behind m on sync, even k behind h0k0 on scalar.
            m_sb = mp.tile([128, KC, D], dt_in, name="m_sb")
            x_sb = {}

            def xtile(h, k):
                t = xp.tile([128, HW], dt_in, tag=f"x{h}{k}", name=f"x{h}{k}")
                x_sb[(h, k)] = t
                return t

            nc.sync.dma_start(out=m_sb[:, 0, :], in_=mm[:, 0, :])
            nc.scalar.dma_start(out=xtile(0, 0)[:], in_=xt0[0])
            nc.sync.dma_start(out=m_sb[:, 1, :], in_=mm[:, 1, :])
            nc.scalar.dma_start(out=m_sb[:, 2:KC, :], in_=mm[:, 2:KC, :])
            nc.sync.dma_start(out=xtile(0, 1)[:], in_=xt0[1])
            nc.scalar.dma_start(out=xtile(0, 2)[:], in_=xt0[2])
            nc.sync.dma_start(out=xtile(0, 3)[:], in_=xt0[3])
            for h in range(1, HB):
                for k in range(KC):
                    eng = nc.scalar if k % 2 == 0 else nc.sync
                    eng.dma_start(
                        out=xtile(h, k)[:],
                        in_=xq[h - 1, :, k * HW : (k + 1) * HW],
                    )

            # output engine per (h,d) tile: gpsimd early (HWDGE rings still
            # pulling inputs), rotate later so no ring serializes
            G, S, C = nc.gpsimd, nc.sync, nc.scalar
            OENG = [
                G, G, G, G,
                G, G, S, C,
                S, C, G, C,
                G, G, S, None,  # last tile handled fine-grained below
            ]

            def copy_eng(j):
                # one PSUM reader per bank: ACT drains j0 banks, DVE j1 banks
                return nc.scalar.copy if j == 0 else nc.vector.tensor_copy

            def emit_mm(pss, h, k, d, j):
                nc.tensor.matmul(
                    pss[d * JH + j][:],
                    m_sb[:, k, d * 128 : (d + 1) * 128],
                    x_sb[(h, k)][:, j * 512 : (j + 1) * 512],
                    start=(k == 0),
                    stop=(k == KC - 1),
                )

            def emit_out(pss, h, d):
                ot = op.tile([128, HW], dt_out, name=f"ot{h}{d}", tag="ot")
                last = h == HB - 1 and d == DT - 1
                if last:
                    # final tile: j0 bank (ACT -> sync) and j1 bank (DVE ->
                    # scalar) drain on parallel engines and rings
                    for j in range(JH):
                        c0 = j * 512
                        copy_eng(j)(ot[:, c0 : c0 + 512],
                                    pss[d * JH + j][:])
                        seng = nc.sync if j == 0 else nc.scalar
                        seng.dma_start(
                            out=yt[h, d, :, c0 : c0 + 512],
                            in_=ot[:, c0 : c0 + 512],
                        )
                else:
                    for j in range(JH):
                        copy_eng(j)(
                            ot[:, j * 512 : (j + 1) * 512],
                            pss[d * JH + j][:],
                        )
                    OENG[h * DT + d].dma_start(out=yt[h, d], in_=ot[:])

            for h in range(HB):
                pss = [
                    pp.tile([128, 512], mybir.dt.float32, tag="ps",
                            name=f"ps_{h}_{dj // JH}_{dj % JH}")
                    for dj in range(DT * JH)
                ]
                if h < HB - 1:
                    # k-outer: quarter starts after just its k0 chunk lands
                    for k in range(KC):
                        for d in range(DT):
                            for j in range(JH):
                                emit_mm(pss, h, k, d, j)
                    for d in range(DT):
                        emit_out(pss, h, d)
                else:
                    # last quarter d-outer: inputs are all resident by now;
                    # each d-group's output streams out 1.7us apart instead
                    # of the whole 1MB bunching after the final MM
                    for d in range(DT):
                        for k in range(KC):
                            for j in range(JH):
                                emit_mm(pss, h, k, d, j)
                        emit_out(pss, h, d)
    nc.compile()
    return nc


def _fold_m(W_v, s_p, W_p, beta_p, W_o):
    """Host-side constant folding of the tiny parameter tensors into M."""
    W_v = np.asarray(W_v, dtype=np.float64)
    s_p = np.asarray(s_p, dtype=np.float64)
    W_p = np.asarray(W_p, dtype=np.float64)
    beta_p = np.asarray(beta_p, dtype=np.float64)
    W_o = np.asarray(W_o, dtype=np.float64)
    phi = np.einsum("h,dhc,d->hc", s_p, W_p, beta_p)
    norma = np.linalg.norm(phi, axis=1)  # [h]
    M = np.einsum("dhc,h,hce->de", W_v, norma, W_o)  # [512, 512]
    return M.astype(np.float32)


_prog_cache = {}
_last_in_maps = None  # kept for test.py profiling reuse
_last_result = None


def _run(in_maps, token, **kwargs):
    if token not in _prog_cache:
        _prog_cache[token] = _build(token)
    return run_bass_kernel_spmd(_prog_cache[token], in_maps, list(range(N_CORES)), **kwargs)


def kernel(x, W_v, s_p, c_p, W_p, W_A, W_o, beta_p, beta_i_p, **_unused):
    global _last_in_maps, _last_result
    token = COMPUTE_DTYPE
    np_dt = _np_dtype(token)

    x = np.asarray(x, dtype=np.float32)
    M = _fold_m(W_v, s_p, W_p, beta_p, W_o)

    # fp16 path: scale M by an exact power of two so M entries and y values
    # sit in fp16 normal range; undo on the host after the run
    out_unscale = 1.0
    if token == "fp16":
        amax = float(np.abs(M).max())
        if amax > 0:
            e = int(np.floor(-np.log2(amax)))
            M = M * np.float32(2.0**e)
            out_unscale = 2.0**-e

    B, N, Dd = x.shape
    assert B * N == ROWS and Dd == D, (x.shape,)

    mmc = np.ascontiguousarray(M.reshape(KC, 128, D).transpose(1, 0, 2)).astype(np_dt)
    xf = x.reshape(ROWS, D)

    in_maps = []
    for c in range(N_CORES):
        sh = xf[c * RPC : (c + 1) * RPC]               # [4096, 512]
        xT = sh.T.astype(np_dt)                        # [512, 4096]
        xr = xT.reshape(KC, 128, HB, HW)               # [k, p, h, c]
        xt0 = np.ascontiguousarray(xr[:, :, 0, :])     # [KC, 128, HW]
        # quarters h1..h3: [128, KC*HW] with k-chunks side by side
        xq = np.ascontiguousarray(
            xr[:, :, 1:, :].transpose(2, 1, 0, 3).reshape(HB - 1, 128, KC * HW)
        )
        in_maps.append({"xt0": xt0, "xq": xq, "mm": mmc})

    _last_in_maps = in_maps
    res = _run(in_maps, token)
    _last_result = res
    out = np.empty((ROWS, D), dtype=np.float32)
    for c in range(N_CORES):
        yb = res.results[c]["yt"].astype(np.float32)   # [HB, DT, 128, HW]
        if out_unscale != 1.0:
            yb *= np.float32(out_unscale)
        # yb[h, d, p, cc] = y[d*128+p, h*HW+cc] -> yc [512, 4096]
        yc = yb.transpose(1, 2, 0, 3).reshape(D, RPC)
        out[c * RPC : (c + 1) * RPC] = yc.T
    return out.reshape(B, N, D)


if __name__ == "__main__":
    # smoke test with random data
    rng = np.random.default_rng(0)
    x = rng.standard_normal((8, 4096, 512)).astype(np.float32)
    W_v = rng.standard_normal((512, 8, 64)).astype(np.float32) * 0.01
    s_p = np.ones((8,), np.float32)
    c_p = np.ones((8,), np.float32)
    W_p = rng.standard_normal((512, 8, 64)).astype(np.float32) * 0.01
    W_A = rng.standard_normal((256, 64)).astype(np.float32)
    W_o = rng.standard_normal((8, 64, 512)).astype(np.float32) * 0.01
    beta_p = rng.standard_normal((512,)).astype(np.float32) * 1e-5
    beta_i_p = rng.standard_normal((4096, 512)).astype(np.float32) * 1e-5
    out = kernel(x, W_v=W_v, s_p=s_p, c_p=c_p, W_p=W_p, W_A=W_A, W_o=W_o,
                 beta_p=beta_p, beta_i_p=beta_i_p)
    M = _fold_m(W_v, s_p, W_p, beta_p, W_o)
    exp = (x.reshape(-1, 512).astype(np.float64) @ M.astype(np.float64)).reshape(8, 4096, 512)
    err = np.abs(out - exp).max() / (np.abs(exp).max() + 1e-30)
    print("smoke rel err:", err)



# revision 1
# speedup vs baseline: 1.2492x; 1.2492x over previous
"""Trainium2 Bass kernel for nn_EstraNet_1443109012284.

Mathematical reduction: the reference's FAVOR+/trig branch (phi_q, aux_q/k,
fr_q/k, aux_A, A) does not feed the output.  The output is exactly

    out[b,n,d] = sum_{h,c} W_o[h,c,d] * norma[h] * sum_{d'} W_v[d',h,c] * x[b,n,d']
               = (x @ M)[b,n,d],   M[d',d] = sum_{h,c} W_v[d',h,c] norma[h] W_o[h,c,d]

with norma[h] = || sum_d s_p[h] W_p[d,h,:] beta_p[d] ||_2.

M is a tiny [512,512] matrix folded on the host; the device does the single
big GEMM  y[32768,512] = x[32768,512] @ M[512,512]  data-parallel over rows:
each of the 8 cores handles 4096 rows.

Device design (per core): compute yT[d, n] = sum_k M[k,d] xT[k,n]
- fp16; loop h(4 quarters) -> k(4 chunks) -> d(4) -> j(2); 8 PSUM banks
  (d,j) per quarter accumulated over k; M pre-scaled by a power of two.
- Measured constraints this schedule is built around:
  * per-HWDGE-ring input rate ~180-250 GB/s, ~360-400 aggregate;
  * DMA completion sem fires ~1.3-4us after last byte (HBM receipt,
    load-dependent) -> first data-gated MM lands ~4.6us;
  * fixed ~2.6us to first DMA packet and a fixed ~7us epilogue
    (255-semaphore clear sweep) on every kernel;
  * PE stream floor 128 x 216ns = 27.7us; any PE idle gap re-throttles
    the HAM clock to half rate.
- m[k0] (sync) and x[h0,k0] (scalar) race as the two leading DMAs, one
  receipt each in parallel; the other m slices and h0 chunks interleave
  across both rings in need order (m1+h0k1+h0k3 on sync, m23+h0k2 on
  scalar); later quarters alternate rings in consumption order.
- Warmup MMs (128-wide, memset-gated) hold the PE from ~1us to the first
  data sem, burning the HAM cold-clock ramp; overshoot costs 107ns each.
- PSUM->SBUF drains split by bank parity: ACT copies j0 banks, DVE j1
  (parallel PSUM readers on different banks are fine on TRN2).
- k-outer for h0-h2 (quarter starts after one chunk), d-outer for the
  last quarter (inputs all resident; spreads the final 1MB of output
  instead of bunching it after the last MM).
- Outputs blocked [h,d,128,1024] (each store DRAM-contiguous); early
  stores on gpsimd SWDGE, later ones on sync/scalar after their input
  queues drain; final tile's two [128,512] halves store on sync and
  scalar in parallel so the tail is one copy + one small store + receipt.
"""

import os as _os
import sys

sys.path.insert(0, "/opt/trn_rl_repo")

import numpy as np

import concourse.bass as bass
import concourse.tile as tile
from concourse import bacc, mybir
from concourse.bass_utils import run_bass_kernel_spmd

N_CORES = 8
ROWS = 32768           # B*N = 8*4096
RPC = ROWS // N_CORES  # rows per core = 4096
D = 512
KC = 4                 # contraction chunks of 128
DT = D // 128          # output row-blocks = 4
HB = 4                 # n-quarters per stripe
HW = RPC // HB         # 1024 columns per quarter
JH = HW // 512         # moving chunks of 512 per quarter = 2

COMPUTE_DTYPE = _os.environ.get("KERNEL_DTYPE", "fp16")
N_WARM = int(_os.environ.get("KERNEL_NWARM", "33"))
WARM_MEMSET = _os.environ.get("KERNEL_WARM_MEMSET", "1") == "1"

_DT = {
    "fp32": mybir.dt.float32,
    "f32r": mybir.dt.float32r,
    "bf16": mybir.dt.bfloat16,
    "fp16": mybir.dt.float16,
}


def _np_dtype(token):
    if token == "bf16":
        import ml_dtypes

        return ml_dtypes.bfloat16
    if token == "fp16":
        return np.float16
    return np.float32


def _build(token):
    dt_in = _DT[token]
    dt_out = mybir.dt.float16 if token == "fp16" else mybir.dt.float32
    nc = bacc.Bacc("TRN2", target_bir_lowering=False)
    xt0 = nc.dram_tensor("xt0", [KC, 128, HW], dt_in, kind="ExternalInput")
    xq = nc.dram_tensor("xq", [HB - 1, 128, KC * HW], dt_in, kind="ExternalInput")
    mm = nc.dram_tensor("mm", [128, KC, D], dt_in, kind="ExternalInput")
    # output blocked [h, d, 128, 1024] so each store is DRAM-contiguous
    yt = nc.dram_tensor("yt", [HB, DT, 128, HW], dt_out, kind="ExternalOutput")

    with tile.TileContext(nc) as tc:
        with (
            tc.tile_pool(name="xp", bufs=1) as xp,
            tc.tile_pool(name="mp", bufs=1) as mp,
            tc.tile_pool(name="op", bufs=4) as op,
            tc.tile_pool(name="pp", bufs=8, space="PSUM") as pp,
        ):
            # PE warmup: 128-wide matmuls (on uninitialized SBUF - contents
            # are irrelevant, the PSUM bank is cleared by the first real MM)
            # bridge the PE from ~0.4us to the first data sem and burn the
            # HAM cold-clock ramp; each overshoot MM costs only ~107ns.
            wz = mp.tile([128, 128], mybir.dt.float16, name="wz")
            if WARM_MEMSET:
                nc.gpsimd.memset(wz[:], 1.0)
            warm = pp.tile([128, 512], mybir.dt.float32, tag="ps", name="warm")
            for w in range(N_WARM):
                nc.tensor.matmul(
                    warm[:, 0:128], wz[:], wz[:], start=True, stop=True
                )

            # two parallel critical DMAs lead the rings: full m (sync) and
            # x[h0,k0] (scalar) — each sem pays one ~2-3us receipt, in
            # parallel.  Then chunks alternate rings in consumption order:
            # odd k behind m on sync, even k behind h0k0 on scalar.
            m_sb = mp.tile([128, KC, D], dt_in, name="m_sb")
            x_sb = {}

            def xtile(h, k):
                t = xp.tile([128, HW], dt_in, tag=f"x{h}{k}", name=f"x{h}{k}")
                x_sb[(h, k)] = t
                return t

            nc.sync.dma_start(out=m_sb[:, 0, :], in_=mm[:, 0, :])
            nc.scalar.dma_start(out=xtile(0, 0)[:], in_=xt0[0])
            nc.sync.dma_start(out=m_sb[:, 1, :], in_=mm[:, 1, :])
            nc.scalar.dma_start(out=m_sb[:, 2:KC, :], in_=mm[:, 2:KC, :])
            nc.sync.dma_start(out=xtile(0, 1)[:], in_=xt0[1])
            nc.scalar.dma_start(out=xtile(0, 2)[:], in_=xt0[2])
            nc.sync.dma_start(out=xtile(0, 3)[:], in_=xt0[3])
            for h in range(1, HB):
                for k in range(KC):
                    eng = nc.scalar if k % 2 == 0 else nc.sync
                    eng.dma_start(
                        out=xtile(h, k)[:],
                        in_=xq[h - 1, :, k * HW : (k + 1) * HW],
                    )

            # output engine per (h,d) tile: gpsimd early (HWDGE rings still
            # pulling inputs), rotate later so no ring serializes
            G, S, C = nc.gpsimd, nc.sync, nc.scalar
            OENG = [
                G, G, G, G,
                G, G, S, C,
                S, C, G, C,
                G, G, S, None,  # last tile handled fine-grained below
            ]

            def copy_eng(j):
                # one PSUM reader per bank: ACT drains j0 banks, DVE j1 banks
                return nc.scalar.copy if j == 0 else nc.vector.tensor_copy

            def emit_mm(pss, h, k, d, j):
                nc.tensor.matmul(
                    pss[d * JH + j][:],
                    m_sb[:, k, d * 128 : (d + 1) * 128],
                    x_sb[(h, k)][:, j * 512 : (j + 1) * 512],
                    start=(k == 0),
                    stop=(k == KC - 1),
                )

            def emit_out(pss, h, d):
                ot = op.tile([128, HW], dt_out, name=f"ot{h}{d}", tag="ot")
                last = h == HB - 1 and d == DT - 1
                if last:
                    # final tile: j0 bank (ACT -> sync) and j1 bank (DVE ->
                    # scalar) drain on parallel engines and rings
                    for j in range(JH):
                        c0 = j * 512
                        copy_eng(j)(ot[:, c0 : c0 + 512],
                                    pss[d * JH + j][:])
                        seng = nc.sync if j == 0 else nc.scalar
                        seng.dma_start(
                            out=yt[h, d, :, c0 : c0 + 512],
                            in_=ot[:, c0 : c0 + 512],
                        )
                else:
                    for j in range(JH):
                        copy_eng(j)(
                            ot[:, j * 512 : (j + 1) * 512],
                            pss[d * JH + j][:],
                        )
                    OENG[h * DT + d].dma_start(out=yt[h, d], in_=ot[:])

            for h in range(HB):
                pss = [
                    pp.tile([128, 512], mybir.dt.float32, tag="ps",
                            name=f"ps_{h}_{dj // JH}_{dj % JH}")
                    for dj in range(DT * JH)
                ]
                if h < HB - 1:
                    # k-outer: quarter starts after just its k0 chunk lands
                    for k in range(KC):
                        for d in range(DT):
                            for j in range(JH):
                                emit_mm(pss, h, k, d, j)
                    for d in range(DT):
                        emit_out(pss, h, d)
                else:
                    # last quarter d-outer: inputs are all resident by now;
                    # each d-group's output streams out 1.7us apart instead
                    # of the whole 1MB bunching after the final MM
                    for d in range(DT):
                        for k in range(KC):
                            for j in range(JH):
                                emit_mm(pss, h, k, d, j)
                        emit_out(pss, h, d)
    nc.compile()
    return nc


def _fold_m(W_v, s_p, W_p, beta_p, W_o):
    """Host-side constant folding of the tiny parameter tensors into M."""
    W_v = np.asarray(W_v, dtype=np.float64)
    s_p = np.asarray(s_p, dtype=np.float64)
    W_p = np.asarray(W_p, dtype=np.float64)
    beta_p = np.asarray(beta_p, dtype=np.float64)
    W_o = np.asarray(W_o, dtype=np.float64)
    phi = np.einsum("h,dhc,d->hc", s_p, W_p, beta_p)
    norma = np.linalg.norm(phi, axis=1)  # [h]
    M = np.einsum("dhc,h,hce->de", W_v, norma, W_o)  # [512, 512]
    return M.astype(np.float32)


_prog_cache = {}
_last_in_maps = None  # kept for test.py profiling reuse
_last_result = None


def _run(in_maps, token, **kwargs):
    if token not in _prog_cache:
        _prog_cache[token] = _build(token)
    return run_bass_kernel_spmd(_prog_cache[token], in_maps, list(range(N_CORES)), **kwargs)


def kernel(x, W_v, s_p, c_p, W_p, W_A, W_o, beta_p, beta_i_p, **_unused):
    global _last_in_maps, _last_result
    token = COMPUTE_DTYPE
    np_dt = _np_dtype(token)

    x = np.asarray(x, dtype=np.float32)
    M = _fold_m(W_v, s_p, W_p, beta_p, W_o)

    # fp16 path: scale M by an exact power of two so M entries and y values
    # sit in fp16 normal range; undo on the host after the run
    out_unscale = 1.0
    if token == "fp16":
        amax = float(np.abs(M).max())
        if amax > 0:
            e = int(np.floor(-np.log2(amax)))
            M = M * np.float32(2.0**e)
            out_unscale = 2.0**-e

    B, N, Dd = x.shape
    assert B * N == ROWS and Dd == D, (x.shape,)

    mmc = np.ascontiguousarray(M.reshape(KC, 128, D).transpose(1, 0, 2)).astype(np_dt)
    xf = x.reshape(ROWS, D)

    in_maps = []
    for c in range(N_CORES):
        sh = xf[c * RPC : (c + 1) * RPC]               # [4096, 512]
        xT = sh.T.astype(np_dt)                        # [512, 4096]
        xr = xT.reshape(KC, 128, HB, HW)               # [k, p, h, c]
        xt0 = np.ascontiguousarray(xr[:, :, 0, :])     # [KC, 128, HW]
        # quarters h1..h3: [128, KC*HW] with k-chunks side by side
        xq = np.ascontiguousarray(
            xr[:, :, 1:, :].transpose(2, 1, 0, 3).reshape(HB - 1, 128, KC * HW)
        )
        in_maps.append({"xt0": xt0, "xq": xq, "mm": mmc})

    _last_in_maps = in_maps
    res = _run(in_maps, token)
    _last_result = res
    out = np.empty((ROWS, D), dtype=np.float32)
    for c in range(N_CORES):
        yb = res.results[c]["yt"].astype(np.float32)   # [HB, DT, 128, HW]
        if out_unscale != 1.0:
            yb *= np.float32(out_unscale)
        # yb[h, d, p, cc] = y[d*128+p, h*HW+cc] -> yc [512, 4096]
        yc = yb.transpose(1, 2, 0, 3).reshape(D, RPC)
        out[c * RPC : (c + 1) * RPC] = yc.T
    return out.reshape(B, N, D)


if __name__ == "__main__":
    # smoke test with random data
    rng = np.random.default_rng(0)
    x = rng.standard_normal((8, 4096, 512)).astype(np.float32)
    W_v = rng.standard_normal((512, 8, 64)).astype(np.float32) * 0.01
    s_p = np.ones((8,), np.float32)
    c_p = np.ones((8,), np.float32)
    W_p = rng.standard_normal((512, 8, 64)).astype(np.float32) * 0.01
    W_A = rng.standard_normal((256, 64)).astype(np.float32)
    W_o = rng.standard_normal((8, 64, 512)).astype(np.float32) * 0.01
    beta_p = rng.standard_normal((512,)).astype(np.float32) * 1e-5
    beta_i_p = rng.standard_normal((4096, 512)).astype(np.float32) * 1e-5
    out = kernel(x, W_v=W_v, s_p=s_p, c_p=c_p, W_p=W_p, W_A=W_A, W_o=W_o,
                 beta_p=beta_p, beta_i_p=beta_i_p)
    M = _fold_m(W_v, s_p, W_p, beta_p, W_o)
    exp = (x.reshape(-1, 512).astype(np.float64) @ M.astype(np.float64)).reshape(8, 4096, 512)
    err = np.abs(out - exp).max() / (np.abs(exp).max() + 1e-30)
    print("smoke rel err:", err)

